# revision 26
# baseline (speedup 1.0000x reference)
"""Trainium2 Bass kernel for the nn_Decoder problem.

Data-parallel over batch: 32 samples -> 8 cores x 4 samples each.
On-device activations are "feature-major" ([D partitions, tokens] free) so
weight matrices act directly as stationary (lhsT) matmul operands.
"""

import numpy as np

import concourse.bass as bass
import concourse.tile as tile
from concourse import bacc, mybir
from concourse import bass_utils

F32 = mybir.dt.float32
P = 128

# model dims
B, N, M, T, S = 32, 20, 400, 21, 8
A = N + 1            # 21 agents
L = A + M            # 421 encoding tokens
D = 256
H = 8
DH = D // H          # 32
R, PP, FF, K = 6, 50, 80, 6
LENV = A + L         # 442
NEG = -1e9
NCORES = 8
NS = B // NCORES     # 4 samples per core
ISQ = float(1.0 / np.sqrt(DH))
TWO_PI = float(2 * np.pi)

AluOp = mybir.AluOpType
Act = mybir.ActivationFunctionType
AxX = mybir.AxisListType.X


def cdiv(a, b):
    return -(-a // b)


def chunks(n, step=P):
    return [(i, i * step, min(step, n - i * step)) for i in range(cdiv(n, step))]


# ----------------------------------------------------------------------------
# device program
# ----------------------------------------------------------------------------

class Builder:
    def __init__(self, tc, ctx):
        self.tc = tc
        self.nc = tc.nc
        self.ctx = ctx
        self.wp = ctx.enter_context(tc.tile_pool(name="wp", bufs=1))
        self.sp = ctx.enter_context(tc.tile_pool(name="sp", bufs=2))
        self.pp = ctx.enter_context(tc.tile_pool(name="pp", bufs=2,
                                                 space="PSUM"))
        self.ev_i = 0
        self.n_i = 0
        self.dram = {}

    def t(self, pool, shape, tag, bufs=None):
        self.n_i += 1
        return pool.tile(list(shape), F32, tag=tag, bufs=bufs,
                         name=f"{tag}_{self.n_i}")

    def din(self, name, shape):
        if name not in self.dram:
            self.dram[name] = self.nc.dram_tensor(
                name, list(shape), F32, kind="ExternalInput").ap()
        return self.dram[name]

    def dout(self, name, shape):
        if name not in self.dram:
            self.dram[name] = self.nc.dram_tensor(
                name, list(shape), F32, kind="ExternalOutput").ap()
        return self.dram[name]

    def wtile(self, name, indim, outdim):
        dr = self.din(name, (indim, outdim))
        ck = cdiv(indim, P)
        t = self.wp.tile([min(P, indim), ck, outdim], F32, tag=name)
        for c, off, sz in chunks(indim):
            self.nc.sync.dma_start(out=t[:sz, c, :], in_=dr[off:off + sz, :])
        return t

    def btile(self, name, n):
        dr = self.din(name, (n,))
        p = min(P, n)
        t = self.wp.tile([p, cdiv(n, P)], F32, tag=name)
        self.nc.sync.dma_start(out=t[:, :], in_=dr.rearrange("(c p) -> p c", p=p))
        return t

    def rowtile(self, name, shape):
        dr = self.din(name, shape)
        t = self.wp.tile([1] + list(shape), F32, tag=name)
        self.nc.sync.dma_start(out=t[:], in_=dr.unsqueeze(0))
        return t

    def evict(self, out, ps, bias=None, relu=False, scale=None, res=None):
        """out = act(ps * scale + bias) [+ res]; bias is per-partition AP."""
        nc = self.nc
        if res is not None:
            if bias is None:
                nc.vector.tensor_tensor(out=out, in0=ps, in1=res, op=AluOp.add)
            else:
                nc.vector.scalar_tensor_tensor(
                    out=out, in0=ps, scalar=bias, in1=res,
                    op0=AluOp.add, op1=AluOp.add)
            return
        if scale is not None:
            nc.scalar.activation(out, ps, Act.Relu if relu else Act.Identity,
                                 bias=0.0 if bias is None else bias,
                                 scale=scale)
            return
        self.ev_i += 1
        if self.ev_i % 2 == 0:
            if bias is None and not relu:
                nc.scalar.copy(out, ps)
            else:
                nc.scalar.activation(out, ps, Act.Relu if relu else Act.Identity,
                                     bias=0.0 if bias is None else bias)
        else:
            if bias is None and not relu:
                nc.vector.tensor_copy(out, ps)
            elif relu:
                nc.vector.tensor_scalar(
                    out=out, in0=ps, scalar1=0.0 if bias is None else bias,
                    scalar2=0.0, op0=AluOp.add, op1=AluOp.max)
            else:
                nc.vector.tensor_scalar_add(out, ps, bias)

    def psum(self, rows, cols, tag="psA"):
        return self.t(self.pp, [rows, cols], tag, bufs=2)

    def mm_acc(self, ps, wt, mslice, xs, start=True, stop=True):
        n = len(xs)
        for i, x in enumerate(xs):
            ksz = x.shape[0]
            self.nc.tensor.matmul(ps, wt[:ksz, i, mslice], x,
                                  start=(start and i == 0),
                                  stop=(stop and i == n - 1))

    def proj_fm(self, wt, xs, n_free, outdim, bias=None, relu=False,
                scale=None, res=None, out_tag="prj", bufs=1):
        out = self.sp.tile([min(P, outdim), cdiv(outdim, P), n_free], F32,
                           tag=out_tag, bufs=bufs)
        for m, moff, msz in chunks(outdim):
            ps = self.psum(P, 512)
            self.mm_acc(ps[:msz, :n_free], wt, slice(moff, moff + msz), xs)
            b = None if bias is None else bias[:msz, m:m + 1]
            r = None if res is None else res[m]
            self.evict(out[:msz, m, :], ps[:msz, :n_free], bias=b, relu=relu,
                       scale=scale, res=r)
        return out


def _emit(b):
    import os
    STOP = int(os.environ.get("KSTAGE", "99"))
    def stop_at(n):
        if n >= STOP:
            raise _StopEmit()
    nc = b.nc
    wp, sp = b.wp, b.sp
    AG = NS * A      # 84 batched agent tokens
    RT = NS * R      # 24 batched ref tokens

    # ---------------- inputs ----------------
    encT = b.din("encT", (NS, D, L))
    csT_dr = b.din("csT", (NS, S, A))
    refT_dr = b.din("refT", (NS, 5, R * PP))

    m_map = b.rowtile("m_map", (NS, M))
    m_act = b.rowtile("m_act", (NS, A))
    m_enc = b.rowtile("m_enc", (NS, L))
    m_env = b.rowtile("m_env", (NS, LENV))
    m_ref = b.rowtile("m_ref", (NS, R))

    eye = wp.tile([P, P], F32, tag="eye")
    nc.sync.dma_start(out=eye[:], in_=b.din("eye128", (P, P)))
    tri = wp.tile([FF, FF], F32, tag="tri")
    nc.sync.dma_start(out=tri[:], in_=b.din("tri80", (FF, FF)))
    ones = wp.tile([8, P], F32, tag="ones")
    nc.sync.dma_start(out=ones[:], in_=b.din("ones8", (8, P)))

    ca_q, ca_k, ca_v, ca_o = (b.wtile(f"ca_W{x}", D, D) for x in "qkvo")
    mm_q, mm_k, mm_v, mm_o = (b.wtile(f"mm_W{x}", D, D) for x in "qkvo")
    it_q, it_k, it_v, it_o = (b.wtile(f"it_W{x}", D, D) for x in "qkvo")
    dl_q, dl_k, dl_v, dl_o = (b.wtile(f"dl_W{x}", D, D) for x in "qkvo")
    fu_W1 = b.wtile("fu_W1", 2 * D, 2 * D); fu_b1 = b.btile("fu_b1", 2 * D)
    fu_W2 = b.wtile("fu_W2", 2 * D, D);     fu_b2 = b.btile("fu_b2", D)
    g_Wt = b.wtile("g_Wt", D, K * FF * 4)
    g_bt = b.rowtile("g_bt", (K * FF * 4,))
    g_Ws = b.wtile("g_Ws", D, K)
    g_bs = b.rowtile("g_bs", (K,))
    fe_Wt = b.wtile("fe_Wt", FF * 2, D);  fe_bt = b.btile("fe_bt", D)
    fe_Wx = b.wtile("fe_Wx", S, D);       fe_bx = b.btile("fe_bx", D)
    fe_Wo = b.wtile("fe_Wo", D, D);       fe_bo = b.btile("fe_bo", D)
    r_W1 = b.wtile("r_W1", 5, D);   r_b1 = b.btile("r_b1", D)
    r_W2 = b.wtile("r_W2", D, D);   r_b2 = b.btile("r_b2", D)
    d_W1 = b.wtile("d_W1", D, D);   d_b1 = b.btile("d_b1", D)
    d_W2 = b.wtile("d_W2", D, D);   d_b2 = b.btile("d_b2", D)
    d_Wsc = b.wtile("d_Wsc", D, 1)
    d_bsc = b.btile("d_bsc", 1)
    dl_qb = b.btile("dl_qbias", D)
    p_W1 = b.wtile("p_W1", D, 512);  p_b1 = b.btile("p_b1", 512)
    p_W2 = b.wtile("p_W2", 512, D);  p_b2 = b.btile("p_b2", D)
    p_W3a = b.wtile("p_W3acc", D, FF); p_b3a = b.btile("p_b3acc", FF)
    p_W3s = b.wtile("p_W3st", D, FF);  p_b3s = b.btile("p_b3st", FF)

    ap_out = b.dout("ap_out", (NS, A, K * FF * 4))
    sc_out = b.dout("sc_out", (NS, A, K))
    plan_out = b.dout("plan_out", (NS, FF, 3))

    # ---------------- env / encoding (feature-major, per sample) ----------
    env = []
    for s in range(NS):
        t = wp.tile([P, 2, LENV], F32, tag=f"env{s}")
        nc.sync.dma_start(out=t[:, :, A:],
                          in_=encT[s].rearrange("(c p) t -> p c t", p=P))
        env.append(t)

    csT = wp.tile([S, NS, A], F32, tag="csT")
    nc.sync.dma_start(out=csT[:], in_=csT_dr.rearrange("s e a -> e s a"))
    refT = wp.tile([5, NS, R * PP], F32, tag="refT")
    nc.sync.dma_start(out=refT[:], in_=refT_dr.rearrange("s e t -> e s t"))

    # ---------------- attention machinery ----------------
    def kv_build(s, xs, nk, wk, wv):
        kt = sp.tile([P, 2, LENV], F32, tag="ktbig", bufs=4)
        for c in range(2):
            ps = b.psum(P, 512)
            b.mm_acc(ps[:, :nk], wk, slice(c * P, c * P + P), xs)
            b.evict(kt[:, c, :nk], ps[:, :nk])
        vv = sp.tile([P, 4, D], F32, tag="vvbig", bufs=4)
        for t_i, toff, tsz in chunks(nk):
            ps = b.psum(P, 512)
            for i, x in enumerate(xs):
                nc.tensor.matmul(ps[:tsz, :D], x[:, toff:toff + tsz],
                                 wv[:, i, :], start=(i == 0), stop=(i == 1))
            b.evict(vv[:tsz, t_i, :], ps[:tsz, :D])
        return kt, vv, nk

    def attn(xq_all, wq, wo, kvs, mask_t, nq, q_bias=None, out_res=None,
             out_tag="attT"):
        nt = NS * nq
        hq = 4 * nq
        # Q projection, batched over samples; fold 1/sqrt(DH) (+ bias)
        qt = sp.tile([P, 2, nt], F32, tag="qt", bufs=1)
        for c in range(2):
            ps = b.psum(P, 512)
            b.mm_acc(ps[:, :nt], wq, slice(c * P, (c + 1) * P), xq_all)
            qb_ap = None if q_bias is None else q_bias[:, c:c + 1]
            nc.scalar.activation(qt[:, c, :], ps[:, :nt], Act.Identity,
                                 bias=0.0 if qb_ap is None else qb_ap,
                                 scale=ISQ)
        avT = sp.tile([P, 2, nt], F32, tag="avT", bufs=1)
        for s in range(NS):
            kt, vv, nk = kvs[s]
            tck = chunks(nk)
            # blocked Q [128, (4 heads, nq)]
            qb = sp.tile([P, 2, hq], F32, tag="qb", bufs=1)
            for c in range(2):
                src = qt[:, c, s * nq:(s + 1) * nq].unsqueeze(1).broadcast_to(
                    [P, 4, nq])
                dst = qb[:, c, :].rearrange("p (g q) -> p g q", g=4)
                nc.gpsimd.affine_select(
                    out=dst, in_=src, compare_op=AluOp.is_ge, fill=0.0,
                    base=0, channel_multiplier=1, pattern=[[-DH, 4], [0, nq]])
                nc.gpsimd.affine_select(
                    out=dst, in_=dst, compare_op=AluOp.is_ge, fill=0.0,
                    base=DH - 1, channel_multiplier=-1,
                    pattern=[[DH, 4], [0, nq]])
            # logits -> exp (no max-sub; masks pre-added via K=1 matmul)
            expw = sp.tile([hq, 2, LENV], F32, tag="expw", bufs=1)
            rs = sp.tile([P, 2], F32, tag="rs")
            rcp = sp.tile([P, 2], F32, tag="rcp")
            diag = sp.tile([P, 2, 84], F32, tag="diag", bufs=1)
            mrow = mask_t[0:1, s, :nk]
            for g in range(2):
                psl = b.psum(hq, 512, tag="psL")
                nc.tensor.matmul(psl[:, :nk], qb[:, g, :], kt[:, g, :nk],
                                 start=True, stop=False)
                nc.tensor.matmul(psl[:hq, :nk], ones[0:1, :hq], mrow,
                                 start=False, stop=True)
                nc.scalar.activation(expw[:, g, :nk], psl[:, :nk], Act.Exp,
                                     accum_out=rs[:hq, g:g + 1])
                nc.vector.reciprocal(rcp[:hq, g:g + 1], rs[:hq, g:g + 1])
                nc.vector.tensor_scalar_mul(diag[:hq, g, :hq], eye[:hq, :hq],
                                            rcp[:hq, g:g + 1])
            # transpose + normalize: wT[key, (group, head, q)]
            wT = sp.tile([P, 4, 2, hq], F32, tag="wT", bufs=1)
            for t_i, toff, tsz in tck:
                for g in range(2):
                    psw = b.psum(P, 168, tag="psW")
                    nc.tensor.matmul(psw[:tsz, :hq],
                                     expw[:hq, g, toff:toff + tsz],
                                     diag[:hq, g, :hq], start=True, stop=True)
                    b.evict(wT[:tsz, t_i, g, :], psw[:tsz, :hq])
            # AV, feature-major out
            for c in range(2):
                pso = b.psum(P, 512, tag="psO")
                for hl in range(4):
                    h = 4 * c + hl
                    for t_i, toff, tsz in tck:
                        nc.tensor.matmul(
                            pso[hl * DH:(hl + 1) * DH, :nq],
                            vv[:tsz, t_i, h * DH:(h + 1) * DH],
                            wT[:tsz, t_i, c, hl * nq:(hl + 1) * nq],
                            start=(t_i == 0), stop=(t_i == len(tck) - 1),
                            tile_position=(0, hl * DH))
                b.evict(avT[:, c, s * nq:(s + 1) * nq], pso[:, :nq])
        # Wo projection (+ optional residual), batched
        out = sp.tile([P, 2, nt], F32, tag=out_tag, bufs=1)
        for m in range(2):
            ps = b.psum(P, 512)
            b.mm_acc(ps[:, :nt], wo, slice(m * P, (m + 1) * P),
                     [avT[:, 0, :], avT[:, 1, :]])
            res = None if out_res is None else out_res[:, m, :]
            b.evict(out[:, m, :], ps[:, :nt], res=res)
        return out

    # ---------------- stage 1: ca cross + self attention ----------------
    agg = wp.tile([P, 2, AG], F32, tag="agg")
    for s in range(NS):
        for c in range(2):
            nc.vector.tensor_copy(agg[:, c, s * A:(s + 1) * A],
                                  env[s][:, c, A:2 * A])

    stop_at(1)
    kv_ca = [kv_build(s, [env[s][:, 0, 2 * A:], env[s][:, 1, 2 * A:]], M,
                      ca_k, ca_v) for s in range(NS)]
    alT = attn([agg[:, 0, :], agg[:, 1, :]], ca_q, ca_o, kv_ca, m_map, A,
               out_tag="alT")
    stop_at(2)
    kv_aa = [kv_build(s, [env[s][:, 0, A:2 * A], env[s][:, 1, A:2 * A]], A,
                      ca_k, ca_v) for s in range(NS)]
    aaT = attn([agg[:, 0, :], agg[:, 1, :]], ca_q, ca_o, kv_aa, m_act, A,
               out_tag="aaT")

    # ---------------- stage 2: fusion MLP ----------------
    stop_at(3)
    xs4 = [alT[:, 0, :], alT[:, 1, :], aaT[:, 0, :], aaT[:, 1, :]]
    f1 = b.proj_fm(fu_W1, xs4, AG, 2 * D, bias=fu_b1, relu=True, out_tag="f1")
    interT = b.proj_fm(fu_W2, [f1[:, i, :] for i in range(4)], AG, D,
                       bias=fu_b2, out_tag="interT")

    # ---------------- stage 3: mm attention ----------------
    stop_at(4)
    kv_mm = [kv_build(s, [alT[:, 0, s * A:(s + 1) * A],
                          alT[:, 1, s * A:(s + 1) * A]], A, mm_k, mm_v)
             for s in range(NS)]
    attT = attn([interT[:, 0, :], interT[:, 1, :]], mm_q, mm_o, kv_mm, m_act,
                A, out_tag="att0")

    # ---------------- stage 4: interaction x3 ----------------
    stop_at(5)
    kv_it = [kv_build(s, [env[s][:, 0, A:], env[s][:, 1, A:]], L, it_k, it_v)
             for s in range(NS)]
    for i in range(3):
        attT = attn([attT[:, 0, :], attT[:, 1, :]], it_q, it_o, kv_it, m_enc,
                    A, out_res=attT, out_tag=f"att{i + 1}")

    stop_at(6)
    # ---------------- stage 5: gmm heads (token-major) ----------------
    NT = 480
    ap_sb = wp.tile([AG, 4, NT], F32, tag="ap_sb")
    for n in range(4):
        ps = b.psum(P, NT)
        for c in range(2):
            nc.tensor.matmul(ps[:AG, :], attT[:, c, :],
                             g_Wt[:, c, n * NT:(n + 1) * NT],
                             start=(c == 0), stop=False)
        nc.tensor.matmul(ps[:AG, :], ones[0:1, :AG],
                         g_bt[0:1, n * NT:(n + 1) * NT], start=False,
                         stop=True)
        b.evict(ap_sb[:, n, :], ps[:AG, :])
    for s in range(NS):
        nc.sync.dma_start(out=ap_out[s], in_=ap_sb[s * A:(s + 1) * A, :, :])

    ps_sc = b.psum(AG, K, tag="psW")
    for c in range(2):
        nc.tensor.matmul(ps_sc[:, :], attT[:, c, :], g_Ws[:, c, :],
                         start=(c == 0), stop=False)
    nc.tensor.matmul(ps_sc[:, :], ones[0:1, :AG], g_bs[0:1, :], start=False,
                     stop=True)
    sc_sb = wp.tile([AG, K], F32, tag="sc_sb")
    nc.scalar.copy(sc_sb[:], ps_sc[:])
    for s in range(NS):
        nc.sync.dma_start(out=sc_out[s], in_=sc_sb[s * A:(s + 1) * A, :])

    # modal softmax, scaled by 1/K for the mean
    sce = sp.tile([AG, K], F32, tag="sce")
    srow = sp.tile([AG, 2], F32, tag="srow")
    nc.scalar.activation(sce[:], ps_sc[:], Act.Exp, accum_out=srow[:, 0:1])
    nc.vector.tensor_scalar_mul(srow[:, 1:2], srow[:, 0:1], float(K))
    srcp = sp.tile([AG, 1], F32, tag="srcp")
    nc.vector.reciprocal(srcp[:], srow[:, 1:2])
    smax = sp.tile([AG, K], F32, tag="smax")
    nc.vector.tensor_scalar_mul(smax[:], sce[:], srcp[:])
    ps_st = b.psum(K, AG, tag="psW")
    nc.tensor.transpose(ps_st[:], smax[:], eye[:AG, :AG])
    smaxT = sp.tile([K, AG], F32, tag="smaxT")
    nc.vector.tensor_copy(smaxT[:], ps_st[:])
    wsel = sp.tile([K, AG, K], F32, tag="wsel", bufs=1)
    nc.gpsimd.affine_select(
        out=wsel[:], in_=smaxT[:, :].unsqueeze(2).broadcast_to([K, AG, K]),
        compare_op=AluOp.is_equal, fill=0.0, base=0, channel_multiplier=1,
        pattern=[[0, AG], [-1, K]])
    ps_wb = b.psum(P, AG * K)
    nc.tensor.matmul(ps_wb[:], ones[0:K, :P],
                     wsel[:].rearrange("k a j -> k (a j)"), start=True,
                     stop=True)
    w_bc = sp.tile([P, AG * K], F32, tag="w_bc", bufs=1)
    nc.scalar.copy(w_bc[:], ps_wb[:])

    stop_at(7)
    # ---------------- stage 6: future encoder ----------------
    trA = wp.tile([P, AG, K], F32, tag="trA")
    trB = wp.tile([DH, AG, K], F32, tag="trB")
    apv = ap_sb[:, :, :].rearrange("t n x -> t (n x)").rearrange(
        "t (k f four) -> t k f four", k=K, four=4)
    trajC = sp.tile([AG, K, FF * 2], F32, tag="trajC", bufs=1)
    nc.vector.tensor_copy(
        trajC[:].rearrange("t k (f x) -> t k f x", x=2), apv[:, :, :, 0:2])
    for k in range(K):
        pst = b.psum(P, AG, tag="psW")
        nc.tensor.transpose(pst[:P, :], trajC[:, k, 0:P], eye[:AG, :AG])
        b.evict(trA[:, :, k], pst[:P, :AG])
        pst2 = b.psum(P, AG, tag="psW")
        nc.tensor.transpose(pst2[:DH, :], trajC[:, k, P:FF * 2],
                            eye[:AG, :AG])
        b.evict(trB[:, :, k], pst2[:DH, :AG])

    seT = sp.tile([P, 2, AG], F32, tag="seT", bufs=1)
    for m in range(2):
        ps = b.psum(P, 512)
        nc.tensor.matmul(ps[:, :AG], fe_Wx[:S, 0, m * P:(m + 1) * P],
                         csT[:, :, :].rearrange("e s a -> e (s a)"),
                         start=True, stop=True)
        b.evict(seT[:, m, :], ps[:, :AG], bias=fe_bx[:, m:m + 1])

    NK6 = AG * K
    fut1 = sp.tile([P, 2, NK6], F32, tag="fut1", bufs=1)
    for m in range(2):
        ps = b.psum(P, 512)
        nc.tensor.matmul(ps[:, :NK6], fe_Wt[:, 0, m * P:(m + 1) * P],
                         trA[:].rearrange("p a k -> p (a k)"), start=True,
                         stop=False)
        nc.tensor.matmul(ps[:, :NK6], fe_Wt[:DH, 1, m * P:(m + 1) * P],
                         trB[:].rearrange("p a k -> p (a k)"), start=False,
                         stop=True)
        se_b = seT[:, m, :].unsqueeze(2).broadcast_to([P, AG, K])
        ps_v = ps[:, :NK6].rearrange("p (a k) -> p a k", k=K)
        nc.vector.tensor_tensor(out=ps_v, in0=ps_v, in1=se_b, op=AluOp.add)
        nc.scalar.activation(fut1[:, m, :], ps[:, :NK6], Act.Relu,
                             bias=fe_bt[:, m:m + 1])
    for m in range(2):
        ps = b.psum(P, 512)
        b.mm_acc(ps[:, :NK6], fe_Wo, slice(m * P, (m + 1) * P),
                 [fut1[:, 0, :], fut1[:, 1, :]])
        futm = sp.tile([P, NK6], F32, tag="futm", bufs=1)
        # (fut + bo) * smax/K ; k-sum then yields futures incl. bo/K term
        nc.vector.scalar_tensor_tensor(
            out=futm[:], in0=ps[:, :NK6], scalar=fe_bo[:, m:m + 1],
            in1=w_bc[:], op0=AluOp.add, op1=AluOp.mult)
        for s in range(NS):
            nc.vector.tensor_reduce(
                out=env[s][:, m, 0:A],
                in_=futm[:, s * A * K:(s + 1) * A * K].rearrange(
                    "p (a k) -> p a k", k=K),
                axis=AxX, op=AluOp.add)

    stop_at(8)
    # ---------------- stage 7: ref path encoder ----------------
    xrm = sp.tile([P, 2, RT], F32, tag="xrm", bufs=1)
    for s in range(NS):
        xrp = sp.tile([P, 2, R * PP], F32, tag="xrp", bufs=1)
        for m in range(2):
            ps = b.psum(P, 512)
            nc.tensor.matmul(ps[:, :R * PP], r_W1[:5, 0, m * P:(m + 1) * P],
                             refT[:, s, :], start=True, stop=True)
            nc.scalar.activation(xrp[:, m, :], ps[:, :R * PP], Act.Relu,
                                 bias=r_b1[:, m:m + 1])
            nc.vector.tensor_reduce(
                out=xrm[:, m, s * R:(s + 1) * R],
                in_=xrp[:, m, :].rearrange("p (r q) -> p r q", r=R),
                axis=AxX, op=AluOp.max)
    xrT = b.proj_fm(r_W2, [xrm[:, 0, :], xrm[:, 1, :]], RT, D, bias=r_b2,
                    out_tag="xr0")

    # ---------------- stage 8: decoder layers x4 ----------------
    stop_at(9)
    kv_dl = [kv_build(s, [env[s][:, 0, :], env[s][:, 1, :]], LENV, dl_k, dl_v)
             for s in range(NS)]
    for i in range(4):
        up = attn([xrT[:, 0, :], xrT[:, 1, :]], dl_q, dl_o, kv_dl, m_env, R,
                  q_bias=dl_qb, out_res=xrT, out_tag=f"xru{i}")
        h1 = b.proj_fm(d_W1, [up[:, 0, :], up[:, 1, :]], RT, D, bias=d_b1,
                       relu=True, out_tag="dh1", bufs=2)
        xrT = b.proj_fm(d_W2, [h1[:, 0, :], h1[:, 1, :]], RT, D, bias=d_b2,
                        res=[up[:, m, :] for m in range(2)],
                        out_tag=f"xr{i + 1}")

    stop_at(10)
    # ---------------- stage 9: score_r, one-hot argmax, ego gather -------
    ps_sr = b.psum(1, RT, tag="psW")
    for c in range(2):
        nc.tensor.matmul(ps_sr[:], d_Wsc[:, c, 0:1], xrT[:, c, :],
                         start=(c == 0), stop=(c == 1))
    scr = sp.tile([1, RT], F32, tag="scr")
    nc.vector.scalar_tensor_tensor(
        out=scr[:], in0=ps_sr[:], scalar=d_bsc[0:1, 0:1],
        in1=m_ref[0:1, :, :].rearrange("o s r -> o (s r)"),
        op0=AluOp.add, op1=AluOp.add)
    mx = sp.tile([1, NS], F32, tag="mx")
    nc.vector.tensor_reduce(out=mx[:],
                            in_=scr[:].rearrange("o (s r) -> o s r", r=R),
                            axis=AxX, op=AluOp.max)
    oh = sp.tile([1, RT], F32, tag="oh")
    nc.vector.tensor_tensor(
        out=oh[:].rearrange("o (s r) -> o s r", r=R),
        in0=scr[:].rearrange("o (s r) -> o s r", r=R),
        in1=mx[:].unsqueeze(2).broadcast_to([1, NS, R]),
        op=AluOp.is_equal)
    ps_oht = b.psum(RT, 8, tag="psW")
    nc.tensor.transpose(ps_oht[:RT, 0:1], oh[:], eye[:1, :1])
    ohT = sp.tile([RT, 1], F32, tag="ohT")
    nc.vector.tensor_copy(ohT[:], ps_oht[:RT, 0:1])
    ohb = sp.tile([RT, NS], F32, tag="ohb")
    nc.gpsimd.affine_select(out=ohb[:], in_=ohT[:].broadcast_to([RT, NS]),
                            compare_op=AluOp.is_ge, fill=0.0, base=0,
                            channel_multiplier=1, pattern=[[-R, NS]])
    nc.gpsimd.affine_select(out=ohb[:], in_=ohb[:], compare_op=AluOp.is_ge,
                            fill=0.0, base=R - 1, channel_multiplier=-1,
                            pattern=[[R, NS]])
    egoT = sp.tile([P, 2, NS], F32, tag="egoT", bufs=1)
    for c in range(2):
        ps_xt = b.psum(RT, P)
        nc.tensor.transpose(ps_xt[:], xrT[:, c, :RT], eye[:P, :P])
        xtok = sp.tile([RT, P], F32, tag="xtok", bufs=1)
        nc.scalar.copy(xtok[:], ps_xt[:])
        ps_e = b.psum(P, NS, tag="psW")
        nc.tensor.matmul(ps_e[:], xtok[:], ohb[:], start=True, stop=True)
        nc.vector.tensor_copy(egoT[:, c, :], ps_e[:])

    stop_at(11)
    # ---------------- stage 10: planner MLP (elu) ----------------
    def elu_evict(out_ap, ps, bias_ap):
        rl = sp.tile([P, NS], F32, tag="elu_r")
        mn = sp.tile([P, NS], F32, tag="elu_m")
        ex = sp.tile([P, NS], F32, tag="elu_e")
        nc.scalar.activation(rl[:], ps, Act.Relu, bias=bias_ap)
        nc.vector.tensor_scalar(out=mn[:], in0=ps, scalar1=bias_ap,
                                scalar2=0.0, op0=AluOp.add, op1=AluOp.min)
        nc.scalar.activation(ex[:], mn[:], Act.Exp)
        nc.vector.scalar_tensor_tensor(out=out_ap, in0=ex[:], scalar=-1.0,
                                       op0=AluOp.add, in1=rl[:],
                                       op1=AluOp.add)

    h1T = sp.tile([P, 4, NS], F32, tag="h1T", bufs=1)
    for m in range(4):
        ps = b.psum(P, 512)
        b.mm_acc(ps[:, :NS], p_W1, slice(m * P, (m + 1) * P),
                 [egoT[:, 0, :], egoT[:, 1, :]])
        elu_evict(h1T[:, m, :], ps[:, :NS], p_b1[:, m:m + 1])
    h2T = sp.tile([P, 2, NS], F32, tag="h2T", bufs=1)
    for m in range(2):
        ps = b.psum(P, 512)
        b.mm_acc(ps[:, :NS], p_W2, slice(m * P, (m + 1) * P),
                 [h1T[:, i, :] for i in range(4)])
        elu_evict(h2T[:, m, :], ps[:, :NS], p_b2[:, m:m + 1])

    stop_at(12)
    # ---------------- stage 11: dynamics ----------------
    from contextlib import ExitStack as _ES
    dyn_ctx = _ES()
    dyn = dyn_ctx.enter_context(b.tc.tile_pool(name="dyn", bufs=1))

    def d_t(name, shape=(FF, NS)):
        return b.t(dyn, list(shape), name)

    h2s = [h2T[:, 0, :], h2T[:, 1, :]]
    ps_a = b.psum(FF, NS, tag="psW")
    b.mm_acc(ps_a[:], p_W3a, slice(0, FF), h2s)
    accC = d_t("accC"); t1 = d_t("t1")
    nc.vector.tensor_scalar(out=t1[:], in0=ps_a[:], scalar1=p_b3a[:, 0:1],
                            scalar2=-5.0, op0=AluOp.add, op1=AluOp.max)
    nc.vector.tensor_scalar_min(accC[:], t1[:], 5.0)
    ps_s = b.psum(FF, NS, tag="psW")
    b.mm_acc(ps_s[:], p_W3s, slice(0, FF), h2s)
    strC = d_t("strC"); t2 = d_t("t2")
    nc.vector.tensor_scalar(out=t2[:], in0=ps_s[:], scalar1=p_b3s[:, 0:1],
                            scalar2=-0.5, op0=AluOp.add, op1=AluOp.max)
    nc.vector.tensor_scalar_min(strC[:], t2[:], 0.5)

    icv = dyn.tile([1, 5, NS], F32, tag="icv")
    nc.sync.dma_start(out=icv[:], in_=csT[0:5, :, 0:1])
    ic = [icv[0:1, j, :] for j in range(5)]
    v0 = d_t("v0", (1, NS)); t3 = d_t("t3", (1, NS)); t4 = d_t("t4", (1, NS))
    nc.vector.tensor_tensor(out=t3[:], in0=ic[3], in1=ic[3], op=AluOp.mult)
    nc.vector.tensor_tensor(out=t4[:], in0=ic[4], in1=ic[4], op=AluOp.mult)
    nc.vector.tensor_tensor(out=t3[:], in0=t3[:], in1=t4[:], op=AluOp.add)
    nc.scalar.activation(v0[:], t3[:], Act.Sqrt)

    ps_v = b.psum(FF, NS, tag="psW")
    nc.tensor.matmul(ps_v[:], tri[:, :], accC[:], start=True, stop=False)
    nc.tensor.matmul(ps_v[:], ones[0:1, :FF], v0[:], start=False, stop=True)
    velT = d_t("velT")
    nc.scalar.activation(velT[:], ps_v[:], Act.Relu)

    dyaw = d_t("dyaw")
    nc.vector.tensor_tensor(out=dyaw[:], in0=strC[:], in1=velT[:],
                            op=AluOp.mult)
    ps_y = b.psum(FF, NS, tag="psW")
    nc.tensor.matmul(ps_y[:], tri[:, :], dyaw[:], start=True, stop=False)
    nc.tensor.matmul(ps_y[:], ones[0:1, :FF], ic[2], start=False, stop=True)
    yraw = d_t("yraw")
    nc.vector.tensor_copy(yraw[:], ps_y[:])

    planT = dyn.tile([FF, NS, 3], F32, tag="planT")
    # fmod(yraw, 2pi) = yraw - trunc(yraw/2pi)*2pi, trunc via the 2^23 trick
    MAGIC = 8388608.0
    yq = d_t("yq")
    nc.scalar.activation(yq[:], ps_y[:], Act.Identity, scale=1.0 / TWO_PI)
    sg = d_t("sg")
    nc.scalar.activation(sg[:], yq[:], Act.Sign)
    aq = d_t("aq")
    nc.scalar.activation(aq[:], yq[:], Act.Abs)
    tr = d_t("tr")
    nc.vector.tensor_scalar(out=tr[:], in0=aq[:], scalar1=MAGIC,
                            scalar2=None, op0=AluOp.add)
    nc.vector.tensor_scalar(out=tr[:], in0=tr[:], scalar1=-MAGIC,
                            scalar2=None, op0=AluOp.add)
    # tr = round(aq); trunc = tr - (tr > aq), indicator via sign+relu
    ddt = d_t("dd")
    nc.vector.tensor_tensor(out=ddt[:], in0=tr[:], in1=aq[:],
                            op=AluOp.subtract)
    sgd = d_t("sgd")
    nc.scalar.activation(sgd[:], ddt[:], Act.Sign)
    nc.vector.tensor_scalar(out=sgd[:], in0=sgd[:], scalar1=0.0,
                            scalar2=None, op0=AluOp.max)
    nc.vector.tensor_tensor(out=tr[:], in0=tr[:], in1=sgd[:],
                            op=AluOp.subtract)
    tsg = d_t("tsg")
    nc.vector.tensor_tensor(out=tsg[:], in0=tr[:], in1=sg[:], op=AluOp.mult)
    nc.vector.scalar_tensor_tensor(out=planT[:, :, 2], in0=tsg[:],
                                   scalar=-TWO_PI, op0=AluOp.mult,
                                   in1=yraw[:], op1=AluOp.add)
    # reduce to [-pi, pi] for sin: subtract 2pi*sign where |yaw| > pi
    sg2 = d_t("sg2")
    nc.scalar.activation(sg2[:], planT[:, :, 2], Act.Sign)
    u1 = d_t("u1")
    nc.scalar.activation(u1[:], planT[:, :, 2], Act.Abs, scale=1.0 / TWO_PI)
    gtp = d_t("gtp")
    nc.vector.tensor_scalar(out=gtp[:], in0=u1[:], scalar1=MAGIC,
                            scalar2=None, op0=AluOp.add)
    nc.vector.tensor_scalar(out=gtp[:], in0=gtp[:], scalar1=-MAGIC,
                            scalar2=None, op0=AluOp.add)
    t6 = d_t("t6")
    nc.vector.tensor_tensor(out=t6[:], in0=gtp[:], in1=sg2[:], op=AluOp.mult)
    yred = d_t("yred")
    nc.vector.scalar_tensor_tensor(out=yred[:], in0=t6[:], scalar=-TWO_PI,
                                   op0=AluOp.mult, in1=planT[:, :, 2],
                                   op1=AluOp.add)
    sh = d_t("sh")
    nc.scalar.activation(sh[:], yred[:], Act.Sin, scale=0.5)
    s2 = d_t("s2")
    nc.vector.tensor_tensor(out=s2[:], in0=sh[:], in1=sh[:], op=AluOp.mult)
    cosv = d_t("cosv")
    nc.vector.tensor_scalar(out=cosv[:], in0=s2[:], scalar1=-2.0, scalar2=1.0,
                            op0=AluOp.mult, op1=AluOp.add)
    uu = d_t("uu")
    nc.scalar.activation(uu[:], s2[:], Act.Sqrt, scale=-1.0, bias=1.0)
    sinv = d_t("sinv")
    nc.vector.scalar_tensor_tensor(out=sinv[:], in0=sh[:], scalar=2.0,
                                   op0=AluOp.mult, in1=uu[:], op1=AluOp.mult)
    dx = d_t("dx"); dy = d_t("dy")
    nc.vector.tensor_tensor(out=dx[:], in0=velT[:], in1=cosv[:], op=AluOp.mult)
    nc.vector.tensor_tensor(out=dy[:], in0=velT[:], in1=sinv[:], op=AluOp.mult)
    ps_x = b.psum(FF, NS, tag="psW")
    nc.tensor.matmul(ps_x[:], tri[:, :], dx[:], start=True, stop=False)
    nc.tensor.matmul(ps_x[:], ones[0:1, :FF], ic[0], start=False, stop=True)
    nc.vector.tensor_copy(planT[:, :, 0], ps_x[:])
    ps_yy = b.psum(FF, NS, tag="psW")
    nc.tensor.matmul(ps_yy[:], tri[:, :], dy[:], start=True, stop=False)
    nc.tensor.matmul(ps_yy[:], ones[0:1, :FF], ic[1], start=False, stop=True)
    nc.scalar.copy(planT[:, :, 1], ps_yy[:])
    for s in range(NS):
        nc.sync.dma_start(out=plan_out[s], in_=planT[:, s, :])
    dyn_ctx.close()


class _StopEmit(Exception):
    pass


def build_program(krep=None):
    from contextlib import ExitStack
    nc = bacc.Bacc("TRN2", target_bir_lowering=False, debug=False,
                   enable_asserts=False)
    import os
    if krep is None:
        krep = int(os.environ.get("KREPEAT", "1"))
    with tile.TileContext(nc) as tc:
        with ExitStack() as ctx:
            b = Builder(tc, ctx)
            try:
                for _ in range(krep):
                    _emit(b)
            except _StopEmit:
                pass
    nc.compile()
    return nc


# ----------------------------------------------------------------------------
# host side
# ----------------------------------------------------------------------------

_CACHE = {}


def _np(x):
    return np.asarray(x, dtype=np.float32)


def make_in_maps(actors, encoding, mask, map_mask, actors_mask, ref_paths,
                 params):
    actors = _np(actors)
    encoding = _np(encoding)
    ref_paths = _np(ref_paths)
    mask = np.asarray(mask)
    map_mask = np.asarray(map_mask)
    actors_mask = np.asarray(actors_mask)

    def addm(m):
        return np.where(m, np.float32(NEG), np.float32(0.0)).astype(np.float32)

    m_enc = addm(mask)
    m_map = addm(map_mask)
    m_act = addm(actors_mask)
    m_env = np.concatenate([m_act, m_enc], axis=1)
    m_ref = addm(np.all(ref_paths == 0, axis=(-1, -2)))

    encT = np.ascontiguousarray(encoding.transpose(0, 2, 1))
    csT = np.ascontiguousarray(actors[:, :, -1, :].transpose(0, 2, 1))
    refT = np.ascontiguousarray(
        ref_paths.reshape(B, R * PP, 5).transpose(0, 2, 1))

    p = {k: {kk: _np(vv) for kk, vv in v.items()} for k, v in params.items()
         if k != "m_pos"}
    m_pos = _np(params["m_pos"])
    consts = {
        "eye128": np.eye(P, dtype=np.float32),
        "tri80": np.triu(np.ones((FF, FF), np.float32)) * np.float32(0.1),
        "ones8": np.ones((8, P), np.float32),
    }
    weights = {
        "fu_W1": p["fusion"]["W1"], "fu_b1": p["fusion"]["b1"],
        "fu_W2": p["fusion"]["W2"], "fu_b2": p["fusion"]["b2"],
        "g_Wt": p["gmm"]["Wt"], "g_bt": p["gmm"]["bt"],
        "g_Ws": p["gmm"]["Ws"], "g_bs": p["gmm"]["bs"],
        "fe_Wt": p["fe"]["Wt"], "fe_bt": p["fe"]["bt"],
        "fe_Wx": p["fe"]["Wx"], "fe_bx": p["fe"]["bx"],
        "fe_Wo": p["fe"]["Wo"], "fe_bo": p["fe"]["bo"],
        "r_W1": p["ref"]["W1"], "r_b1": p["ref"]["b1"],
        "r_W2": p["ref"]["W2"], "r_b2": p["ref"]["b2"],
        "d_W1": p["dlm"]["W1"], "d_b1": p["dlm"]["b1"],
        "d_W2": p["dlm"]["W2"], "d_b2": p["dlm"]["b2"],
        "d_Wsc": p["dlm"]["Wsc"], "d_bsc": p["dlm"]["bsc"],
        "p_W1": p["plan"]["W1"], "p_b1": p["plan"]["b1"],
        "p_W2": p["plan"]["W2"], "p_b2": p["plan"]["b2"],
    }
    for nm in ("ca", "mm", "it", "dl"):
        for x in "qkvo":
            weights[f"{nm}_W{x}"] = p[nm][f"W{x}"]
    W3 = p["plan"]["W3"]
    b3 = p["plan"]["b3"]
    weights["p_W3acc"] = np.ascontiguousarray(W3[:, 0::2])
    weights["p_W3st"] = np.ascontiguousarray(W3[:, 1::2])
    weights["p_b3acc"] = np.ascontiguousarray(b3[0::2])
    weights["p_b3st"] = np.ascontiguousarray(b3[1::2])
    weights["dl_qbias"] = (m_pos[0, 0] @ p["dl"]["Wq"]) * np.float32(ISQ)

    in_maps = []
    for c in range(NCORES):
        sl = slice(c * NS, (c + 1) * NS)
        im = {
            "encT": encT[sl], "csT": csT[sl], "refT": refT[sl],
            "m_map": m_map[sl], "m_act": m_act[sl], "m_enc": m_enc[sl],
            "m_env": m_env[sl], "m_ref": m_ref[sl],
        }
        im.update(weights)
        im.update(consts)
        im = {k: np.ascontiguousarray(v, dtype=np.float32)
              for k, v in im.items()}
        in_maps.append(im)
    return in_maps


def get_program():
    if "nc" not in _CACHE:
        _CACHE["nc"] = build_program()
    return _CACHE["nc"]


def assemble(results):
    ap = np.concatenate([r["ap_out"] for r in results], axis=0)
    agents_pred = ap.reshape(B, A, K, FF, 4)
    scores = np.concatenate([r["sc_out"] for r in results], axis=0)
    ego_plan = np.concatenate([r["plan_out"] for r in results], axis=0)
    return agents_pred, scores, ego_plan


def kernel(actors, encoding, mask, map_mask, actors_mask, ref_paths, params):
    nc = get_program()
    in_maps = make_in_maps(actors, encoding, mask, map_mask, actors_mask,
                           ref_paths, params)
    res = bass_utils.run_bass_kernel_spmd(nc, in_maps,
                                          core_ids=list(range(NCORES)))
    return assemble(res.results)
